# revision 32
# baseline (speedup 1.0000x reference)
"""Trainium2 Bass kernel for nn_COVID19linear.

Math (see reference):
    B, A, H  = dense [n, n] scatter-add of (rows, cols, *_nonzero)
    Csum     = C[0:154] + C[1:155]          (sum over the p=2 lags; B identical per lag)
    C_hat    = Csum @ B + mob_c + upsilon @ cov
    D_hat    = Csum @ H + Dsum @ A + mob_d + zeta @ cov
    mob_c[t] = sum_{k,tau} mu[k,tau] * M[k, t+tau]   (nu for mob_d)

Distribution: tensor-parallel, column-shard the three dense matrices over the
8 cores (393 columns each). Each core computes its 393 output columns for all
154 timesteps; host concatenates. The county dim lives on SBUF partitions
(transposed orientation), so all time shifts are free-dim slices.

Lag trick: (C[0:154]+C[1:155]) @ B = G[0:154] + G[1:155] with G = C @ B, so
GEMMs run on raw C^T/D^T with a moving dim of 155 and the lag sum happens once
on the output. The covariate term (constant in t) would be doubled by the
output shift-add, so the host scales upsilon/zeta by 0.5.

Sparsity trick: the three matrices have only ~31440 nonzeros in 3144^2 slots
(0.3% dense), so shipping them dense wastes ~99.7% of the DMA bytes. The B
shard is still DMA'd dense (it feeds the PE earliest), but the H and A shards
are BUILT ON DEVICE by the otherwise-idle GPSIMD engine: the host packs each
shard's nonzeros into per-partition (index, value) lists and gpsimd
local_scatter materializes the dense weight tiles in SBUF (~1.9us per
5-k-tile call). H's last chunk comes dense via DMA instead, balancing the
gpsimd chain against the DMA stream. A tiny no-dep dummy scatter runs first
so the ~2.5us local_scatter ucode library load overlaps the preamble. This
cuts per-core HBM traffic from ~10.9 MB to ~6.5 MB.

Engine economics (measured): DMA stream ~18us across two HWDGE queues, PE
~21us (283 matmuls; warmup matmuls pre-ramp the clock out of its 0.65/1.2GHz
p-states), gpsimd 9 scatters ~17.5us, DVE mob+finalize ~15us with the mob_d
multiplies offloaded to the Activation engine (scalar_tensor_tensor has no
fast DVE uops, but tensor_tensor bf16 runs 2x). DMA triggers are split
across the two HWDGE engines (Sync and Activation) and DMA count is kept to
~16 because the tile framework recycles only ~9 DMA semaphores (a trigger on
a recycled sem stalls until the prior DMA on it fully completes).

Device layout (per core), all bf16 except noted:
    wc [128, 25, 384]       = B shard cols 0:384 re-tiled (3144 pad 3200)
    wcd3 [128, 25, 41]      = remainder cols: B 384:393 at 0:9, H 384:393 at 32:41
    wd4 [128, 5, 384]       = H shard k-tiles 20:25 (dense DMA)
    hdr [128, 504] int16    = scatter idx | scatter val | mu/nu scalars (bf16 views)
    t_wd/t_wa (SBUF only)   = H cols 0:384 / A cols 0:394, gpsimd-scattered
    ct/dt [128, 25, 156]    = C^T / D^T re-tiled (replicated)
    ms [128, 6, 4, 156]     = M shard, county m = q*128 + p  (q<4 padded)
    uzcv [10, 155+155+393]  = 0.5*upsilon bcast | 0.5*zeta bcast | cov shard
    oc/od [512, 154]        = C_hat^T / D_hat^T shard (rows 393+ are pad)

PSUM: one [128, 3, 155] fp32 tile per output for q0..2 (fits one 2KB bank) so
the lag shift-add finalize is 2 wide DVE ops; start=True clears has_written
for the WHOLE bank, so each bank gets exactly ONE start (its first matmul) and
every other first-write relies on the per-element has_written bit.
"""

import sys

if "/opt/trn_rl_repo" not in sys.path:
    sys.path.insert(0, "/opt/trn_rl_repo")

import ml_dtypes
import numpy as np

import concourse.bass as bass  # noqa: F401  (registers types)
import concourse.mybir as mybir
import concourse.tile as tile
from concourse import bacc
from concourse.bass_utils import run_bass_kernel_spmd


def _harden_trace_path():
    """If the caller sets BASS_TRACE / trace=True, run_bass_kernel_spmd under
    axon needs antenv.axon_hooks (absent on this image) and a working artifact
    upload. Install a best-effort NTFF hook and make upload failures
    non-fatal so tracing degrades instead of crashing the kernel."""
    import types

    try:
        import antenv.axon_hooks  # noqa: F401
    except ImportError:
        mod = types.ModuleType("antenv.axon_hooks")
        state = {"hook": None}
        mod.set_axon_ntff_profile_hook = lambda h: state.__setitem__("hook", h)
        mod.get_axon_ntff_profile_hook = lambda: state["hook"]
        sys.modules["antenv.axon_hooks"] = mod
        try:
            import antenv

            antenv.axon_hooks = mod
        except ImportError:
            pass
        try:
            if "/root/.axon_site" not in sys.path:
                sys.path.insert(0, "/root/.axon_site")
            from trn_agent_boot.trn_boot import _ntff_profile_via_ctypes

            hook = _ntff_profile_via_ctypes("/opt/axon/libaxon_pjrt.so")
            if hook is not None:
                mod.set_axon_ntff_profile_hook(hook)
        except Exception:
            pass

    import concourse.bass_utils as _bu

    if not getattr(_bu.upload_artifacts, "_safe", False):
        _orig = _bu.upload_artifacts

        def _safe_upload(tmpdir):
            try:
                return _orig(tmpdir)
            except Exception:
                return f"local:{tmpdir}"

        _safe_upload._safe = True
        _bu.upload_artifacts = _safe_upload


_harden_trace_path()

N = 3144
T = 156
TP = 154
TG = 155  # GEMM moving dim: output before the lag shift-add
NSH = 8
NCOL = N // NSH  # 393
KT = 25  # k tiles of 128 rows for the county dim (3144 padded to 3200)
NMOB = 6
NCOV = 10
MQ = 4  # m sub-blocks of 128 per shard (393 -> 4 blocks, last has 9 rows)
CHUNK = 5  # k-tiles per GEMM chunk (also the scatter-call granularity)
NCH = KT // CHUNK  # 5 chunks
WCW = 384  # wc width: B cols 0:384 (remainder 384:393 lives in wcd3)
WDW = 384  # t_wd width: H cols 0:384 (remainder in wcd3)
WAW = 394  # t_wa width: A cols 0:393 plus one pad col (even for local_scatter)
SCAT_WD = 3  # H chunks built by gpsimd (chunks 3-4 come dense via DMA instead)
NIDX_MIN = 24  # scatter list capacity per (partition, chunk); grows if needed
NWARM = 13  # PE warmup matmuls (pre-ramp the clock p-state, bridge to ~10us)
BF16 = ml_dtypes.bfloat16

F32 = mybir.dt.float32
BF = mybir.dt.bfloat16
I16 = mybir.dt.int16
MULT = mybir.AluOpType.mult
ADD = mybir.AluOpType.add
COPY = mybir.ActivationFunctionType.Copy

_PROGS = {}


def _mwidth(q):
    return min(128, NCOL - q * 128)


def _hdr_width(nidx):
    return 2 * 2 * NCH * nidx + NMOB * 2 * 2 * 2  # scalars are f32 (2 words)


def _build_program(nidx):
    nc = bacc.Bacc(None, target_bir_lowering=False)

    IDXSZ = 2 * NCH * nidx  # int16 words of scatter indices (m=0: H, m=1: A)
    HW = _hdr_width(nidx)

    wc = nc.dram_tensor("wc", [128, KT, WCW], BF, kind="ExternalInput")
    wcd3 = nc.dram_tensor("wcd3", [128, KT, 41], BF, kind="ExternalInput")
    wd4 = nc.dram_tensor("wd4", [128, KT - SCAT_WD * CHUNK, WDW], BF,
                         kind="ExternalInput")
    hdr = nc.dram_tensor("hdr", [128, HW], I16, kind="ExternalInput")
    ct = nc.dram_tensor("ct", [128, KT, T], BF, kind="ExternalInput")
    dt = nc.dram_tensor("dt", [128, KT, T], BF, kind="ExternalInput")
    ms = nc.dram_tensor("ms", [128, NMOB, MQ, T], BF, kind="ExternalInput")
    uzcv = nc.dram_tensor("uzcv", [NCOV, 2 * TG + NCOL], BF, kind="ExternalInput")
    # padded to 512 rows = [128, 4, 154] exactly -> one DMA per output
    oc = nc.dram_tensor("oc", [MQ * 128, TP], BF, kind="ExternalOutput")
    od = nc.dram_tensor("od", [MQ * 128, TP], BF, kind="ExternalOutput")

    with tile.TileContext(nc) as tc:
        with (
            tc.tile_pool(name="big", bufs=1) as big,
            tc.tile_pool(name="psum", bufs=1, space="PSUM") as psum,
        ):
            t_ct = big.tile([128, KT, T], BF, tag="ct")
            t_dt = big.tile([128, KT, T], BF, tag="dt")
            t_ms = big.tile([128, NMOB, MQ, T], BF, tag="ms")
            t_uzcv = big.tile([NCOV, 2 * TG + NCOL], BF, tag="uzcv")
            t_hdr = big.tile([128, HW], I16, tag="hdr")
            t_wc = big.tile([128, KT, WCW], BF, tag="wc")
            t_wcd3 = big.tile([128, KT, 41], BF, tag="wcd3")
            t_wd = big.tile([128, KT, WDW], BF, tag="wd")
            t_wa = big.tile([128, KT, WAW], BF, tag="wa")
            t_mc = big.tile([128, MQ, TP], BF, tag="mc")
            t_md = big.tile([128, MQ, TP], BF, tag="md")
            t_mtmp = big.tile([128, 4, MQ, TP], BF, tag="mtmp")
            t_tmp = big.tile([128, 2, MQ, TP], F32, tag="tmp")
            t_oc = big.tile([128, MQ, TP], BF, tag="oc")
            t_od = big.tile([128, MQ, TP], BF, tag="od")
            t_warm = big.tile([128, 160], BF, tag="warm")
            t_dumd = big.tile([16, 2], BF, tag="dumd")
            t_dumo = big.tile([16, 2], BF, tag="dumo")
            t_dumi = big.tile([16, 2], I16, tag="dumi")

            def sidx(m, c):
                return t_hdr[:, (m * NCH + c) * nidx : (m * NCH + c + 1) * nidx]

            def sval(m, c):
                off = IDXSZ + (m * NCH + c) * nidx
                return t_hdr[:, off : off + nidx].bitcast(BF)

            def scal(k, tau, c):
                off = 2 * IDXSZ + ((k * 2 + tau) * 2 + c) * 2
                return t_hdr[:, off : off + 2].bitcast(F32)

            def chunks():
                for lo in range(0, KT, CHUNK):
                    yield lo // CHUNK, lo, lo + CHUNK

            # --- Sync trigger stream: scatter lists + mob scalars first
            # (gpsimd + the long DVE mob chain are long poles), M next, D^T
            # last (A runs after B per chunk anyway).
            # --- PE's first gating pair (wc0 on Sync, ct0 on Act) is split
            # ACROSS the two queues so it gets the aggregate bandwidth (one
            # busy queue only sees ~half). Then each queue feeds its longest
            # dependent chain by need-time: hdr -> gpsimd scatters, ms halves
            # bracketing dt0 -> DVE mob vs the A GEMM, wc/ct -> PE.
            nc.sync.dma_start(t_wc[:, 0:CHUNK, :], wc[:, 0:CHUNK, :])
            nc.sync.dma_start(t_hdr[:], hdr[:])
            nc.sync.dma_start(t_ms[:, 0:2, :, :], ms[:, 0:2, :, :])
            nc.sync.dma_start(t_dt[:, 0:10, :], dt[:, 0:10, :])
            nc.sync.dma_start(t_ms[:, 2:NMOB, :, :], ms[:, 2:NMOB, :, :])
            nc.sync.dma_start(t_dt[:, 10:KT, :], dt[:, 10:KT, :])

            nc.scalar.dma_start(t_ct[:, 0:CHUNK, :], ct[:, 0:CHUNK, :])
            nc.scalar.dma_start(t_wcd3[:], wcd3[:])
            nc.scalar.dma_start(t_wc[:, CHUNK:15, :], wc[:, CHUNK:15, :])
            nc.scalar.dma_start(t_ct[:, CHUNK:15, :], ct[:, CHUNK:15, :])
            nc.scalar.dma_start(t_wc[:, 15:KT, :], wc[:, 15:KT, :])
            nc.scalar.dma_start(t_ct[:, 15:KT, :], ct[:, 15:KT, :])
            nc.scalar.dma_start(t_wd[:, SCAT_WD * CHUNK : KT, :], wd4[:])
            nc.scalar.dma_start(t_uzcv[:], uzcv[:])

            # --- GPSIMD builds the H and A weight shards from the nonzero
            # lists. Dummy no-dep call first to front-load the ucode library
            # load; then alternate A/H so both GEMMs unblock chunk-by-chunk.
            nc.gpsimd.memset(t_dumd[:], 0)
            nc.gpsimd.memset(t_dumi[:], -1)
            nc.gpsimd.local_scatter(
                t_dumo[:], t_dumd[:], t_dumi[:],
                channels=16, num_elems=2, num_idxs=2,
            )
            for c, lo, hi in chunks():
                if c < SCAT_WD:
                    nc.gpsimd.local_scatter(
                        t_wd[:, lo:hi, :], sval(0, c), sidx(0, c),
                        channels=128, num_elems=CHUNK * WDW, num_idxs=nidx,
                    )
                nc.gpsimd.local_scatter(
                    t_wa[:, lo:hi, :], sval(1, c), sidx(1, c),
                    channels=128, num_elems=CHUNK * WAW, num_idxs=nidx,
                )

            # --- mobility terms. scalar_tensor_tensor has no fast DVE uops,
            # so mob_d's multiplies run on the otherwise-idle Act engine and
            # DVE folds them in with 2x-mode tensor_tensor adds; mob_c stays
            # as a fused STT chain on DVE. k-ordered so the two ms DMA halves
            # unblock the chain progressively.
            nc.vector.memset(t_warm[:], 0)
            si = 0
            for k in range(NMOB):
                for tau in range(2):
                    src = t_ms[:, k, :, tau : tau + TP]
                    if k == 0 and tau == 0:
                        nc.vector.tensor_scalar_mul(t_mc[:], src, scal(k, tau, 0))
                        nc.scalar.mul(t_md[:], src, scal(k, tau, 1))
                        continue
                    if k < 4:
                        nc.vector.scalar_tensor_tensor(
                            t_mc[:], src, scal(k, tau, 0), t_mc[:], MULT, ADD
                        )
                    else:
                        slot = t_mtmp[:, si % 4, :, :]
                        si += 1
                        nc.scalar.mul(slot, src, scal(k, tau, 0))
                        nc.vector.tensor_tensor(t_mc[:], t_mc[:], slot, ADD)
                    slot = t_mtmp[:, si % 4, :, :]
                    si += 1
                    nc.scalar.mul(slot, src, scal(k, tau, 1))
                    nc.vector.tensor_tensor(t_md[:], t_md[:], slot, ADD)

            # --- GEMMs. q0..2 accumulate into one 3-wide PSUM tile per
            # output (single bank) so finalize is 2 wide DVE ops.
            p_cb = psum.tile([128, 3, TG], F32, tag="pcb", name="pcb")
            p_db = psum.tile([128, 3, TG], F32, tag="pdb", name="pdb")
            p_cd3 = psum.tile([41, TG], F32, tag="pcd3", name="pcd3")
            p_d3 = psum.tile([9, TG], F32, tag="pd3", name="pd3")
            p_warm = psum.tile([128, TG], F32, tag="pwarm", name="pwarm")

            def msl(q):
                return slice(q * 128, q * 128 + _mwidth(q))

            def cov_slice(q):
                return t_uzcv[:, 2 * TG + q * 128 : 2 * TG + q * 128 + _mwidth(q)]

            # PE warmup on zeros: ~2.5us of junk matmuls so the clock ramp
            # (0.65 -> 1.2 -> 2.4 GHz after 3us continuous) finishes before
            # the real stream arrives instead of during it.
            for i in range(NWARM):
                nc.tensor.matmul(
                    p_warm[:], t_warm[:, 0:128], t_warm[:, 0:TG],
                    start=(i == 0), stop=(i == NWARM - 1),
                )

            def emit_cd3(lo, hi, last):
                for k in range(lo, hi):
                    nc.tensor.matmul(
                        p_cd3[:], t_wcd3[:, k, :], t_ct[:, k, 0:TG],
                        start=(k == 0), stop=False,
                    )
                if last:
                    nc.tensor.matmul(
                        p_cd3[0:9, :], cov_slice(3), t_uzcv[:, 0:TG],
                        start=False, stop=False,
                    )
                    nc.tensor.matmul(
                        p_cd3[32:41, :], cov_slice(3), t_uzcv[:, TG : 2 * TG],
                        start=False, stop=True,
                    )

            for c, lo, hi in chunks():
                last = hi == KT
                for q in range(3):
                    for k in range(lo, hi):
                        nc.tensor.matmul(
                            p_cb[:, q, :], t_wc[:, k, msl(q)], t_ct[:, k, 0:TG],
                            start=(k == 0 and q == 0), stop=False,
                        )
                if last:
                    # B covs immediately: C q0..2 finalize overlaps A's tail
                    for q in range(3):
                        nc.tensor.matmul(
                            p_cb[:, q, :], cov_slice(q), t_uzcv[:, 0:TG],
                            start=False, stop=True,
                        )
                # cd3 rides after B (wcd3 arrives early on the Act queue)
                emit_cd3(lo, hi, last)

                def emit_h():
                    for q in range(3):
                        for k in range(lo, hi):
                            nc.tensor.matmul(
                                p_db[:, q, :], t_wd[:, k, msl(q)], t_ct[:, k, 0:TG],
                                start=(k == 0 and q == 0), stop=False,
                            )

                # H before A normally (the wd scatters run first per chunk);
                # in the LAST chunk A+d3 go first so p_d3's stop fires while
                # H still runs and the serial D-q3 finalize chain overlaps PE
                if not last:
                    emit_h()
                for q in range(3):
                    for k in range(lo, hi):
                        nc.tensor.matmul(
                            p_db[:, q, :], t_wa[:, k, msl(q)], t_dt[:, k, 0:TG],
                            start=False, stop=False,
                        )
                for k in range(lo, hi):
                    nc.tensor.matmul(
                        p_d3[:], t_wa[:, k, 384:NCOL], t_dt[:, k, 0:TG],
                        start=(k == 0), stop=(k == KT - 1),
                    )
                if last:
                    emit_h()
                    for q in range(3):
                        nc.tensor.matmul(
                            p_db[:, q, :], cov_slice(q), t_uzcv[:, TG : 2 * TG],
                            start=False, stop=True,
                        )

            # --- finalize: out = p[0:154] + p[1:155] + mob. DVE may read
            # PSUM through one operand per op -> two chained ops, batched
            # over q0..2 thanks to the 3-wide PSUM tiles.
            mw3 = _mwidth(3)
            tmpc = t_tmp[:, 0, 0:3, :]
            nc.vector.scalar_tensor_tensor(
                tmpc, p_cb[:, :, 0:TP], 1.0, t_mc[:, 0:3, :], MULT, ADD
            )
            nc.vector.scalar_tensor_tensor(
                t_oc[:, 0:3, :], p_cb[:, :, 1 : TP + 1], 1.0, tmpc, MULT, ADD
            )
            tmp3c = t_tmp[:mw3, 0, 3, :]
            nc.vector.scalar_tensor_tensor(
                tmp3c, p_cd3[0:9, 0:TP], 1.0, t_mc[:mw3, 3, :], MULT, ADD
            )
            nc.vector.scalar_tensor_tensor(
                t_oc[:mw3, 3, :], p_cd3[0:9, 1 : TP + 1], 1.0, tmp3c, MULT, ADD
            )
            nc.scalar.dma_start(
                oc[:].rearrange("(q p) t -> p q t", p=128), t_oc[:]
            )

            # D q3 = shift(p_d3 A-part) + shift(p_cd3 H-part) + mob, first so
            # its 4-op serial chain overlaps the q0..2 cov stops.
            tmp3d = t_tmp[:mw3, 1, 3, :]
            nc.vector.scalar_tensor_tensor(
                tmp3d, p_d3[:, 0:TP], 1.0, t_md[:mw3, 3, :], MULT, ADD
            )
            nc.vector.scalar_tensor_tensor(
                tmp3d, p_d3[:, 1 : TP + 1], 1.0, tmp3d, MULT, ADD
            )
            nc.vector.scalar_tensor_tensor(
                tmp3d, p_cd3[32:41, 0:TP], 1.0, tmp3d, MULT, ADD
            )
            nc.vector.scalar_tensor_tensor(
                t_od[:mw3, 3, :], p_cd3[32:41, 1 : TP + 1], 1.0, tmp3d, MULT, ADD
            )
            tmpd = t_tmp[:, 1, 0:3, :]
            nc.vector.scalar_tensor_tensor(
                tmpd, p_db[:, :, 0:TP], 1.0, t_md[:, 0:3, :], MULT, ADD
            )
            nc.vector.scalar_tensor_tensor(
                t_od[:, 0:3, :], p_db[:, :, 1 : TP + 1], 1.0, tmpd, MULT, ADD
            )
            nc.sync.dma_start(
                od[:].rearrange("(q p) t -> p q t", p=128), t_od[:]
            )

    nc.compile()
    return nc


def _get_program(nidx):
    if nidx not in _PROGS:
        _PROGS[nidx] = _build_program(nidx)
    return _PROGS[nidx]


def _retile_rows(x, pad_rows):
    """[R, F] -> [128, R_pad/128, F], row r = (tile k, partition r - 128k)."""
    r, f = x.shape
    out = np.zeros((pad_rows, f), x.dtype)
    out[:r] = x
    return np.ascontiguousarray(
        out.reshape(pad_rows // 128, 128, f).transpose(1, 0, 2)
    )


def _pack_scatter(ur, uc, uv, width, nidx_cap):
    """Pack one core-shard's nonzeros (global rows ur, local cols uc < width,
    values uv) into per-(chunk, partition) local_scatter lists.

    Returns (idx [128, NCH, nidx], val [128, NCH, nidx], needed) with idx
    padded by -1. `needed` is the true max list length (may exceed nidx_cap,
    in which case the caller must rebuild with a larger capacity)."""
    ktile = ur // 128
    p = ur % 128
    ch = ktile // CHUNK
    kl = ktile % CHUNK
    sidx = (kl * width + uc).astype(np.int64)
    cell = ch * 128 + p
    order = np.argsort(cell, kind="stable")
    cs = cell[order]
    pos = np.zeros(len(cs), np.int64)
    if len(cs):
        starts = np.zeros(len(cs), np.int64)
        first = np.flatnonzero(np.r_[True, cs[1:] != cs[:-1]])
        starts[first] = first
        np.maximum.accumulate(starts, out=starts)
        pos = np.arange(len(cs)) - starts
    needed = int(pos.max()) + 1 if len(cs) else 0
    idx = np.full((NCH * 128, nidx_cap), -1, np.int16)
    val = np.zeros((NCH * 128, nidx_cap), np.float32)
    if len(cs):
        keep = pos < nidx_cap  # caller rebuilds if any dropped
        idx[cs[keep], pos[keep]] = sidx[order][keep]
        val[cs[keep], pos[keep]] = uv[order][keep]
    idx = idx.reshape(NCH, 128, nidx_cap).transpose(1, 0, 2)
    val = val.reshape(NCH, 128, nidx_cap).transpose(1, 0, 2)
    return idx, val, needed


def _host_inputs(C, D, M, cov, B_nonzero, A_nonzero, H_nonzero, mu, nu,
                 upsilon, zeta, rows, cols, nidx_cap):
    rows = np.asarray(rows).astype(np.int64)
    cols = np.asarray(cols).astype(np.int64)

    # dedupe-sum the shared sparsity pattern once (local_scatter forbids
    # duplicate indices; torch .to_dense() semantics sum duplicates)
    key = rows * N + cols
    order = np.argsort(key, kind="stable")
    ks = key[order]
    uniq = np.r_[True, ks[1:] != ks[:-1]]
    gid = np.cumsum(uniq) - 1
    ukey = ks[uniq]
    ur_all = (ukey // N).astype(np.int64)
    uc_all = (ukey % N).astype(np.int64)
    uvals = {}
    for name, vn in (("B", B_nonzero), ("A", A_nonzero), ("H", H_nonzero)):
        v = np.asarray(vn, np.float32)[order]
        uv = np.zeros(len(ukey), np.float32)
        np.add.at(uv, gid, v)
        uvals[name] = uv

    Bd = np.zeros((N, N), np.float32)
    Bd[ur_all, uc_all] = uvals["B"]

    ct = _retile_rows(np.ascontiguousarray(np.asarray(C, np.float32).T), KT * 128)
    dt = _retile_rows(np.ascontiguousarray(np.asarray(D, np.float32).T), KT * 128)
    ct = ct.astype(BF16)
    dt = dt.astype(BF16)

    # the output lag shift-add doubles the (t-constant) covariate term
    uz = np.zeros((NCOV, 2 * TG + NCOL), np.float32)
    uz[:, 0:TG] = 0.5 * np.asarray(upsilon, np.float32)[:, None]
    uz[:, TG : 2 * TG] = 0.5 * np.asarray(zeta, np.float32)[:, None]

    # mu/nu as f32 scalars, [k, tau, c] flattened, bcast down partitions
    munu = np.stack([np.asarray(mu, np.float32), np.asarray(nu, np.float32)], -1)
    scb = np.tile(munu.reshape(1, -1).astype(np.float32), (128, 1))

    covf = np.asarray(cov, np.float32)
    Mf = np.asarray(M, np.float32)

    in_maps = []
    needed_max = 0
    for j in range(NSH):
        sh = slice(j * NCOL, (j + 1) * NCOL)
        m = {"ct": ct, "dt": dt}
        m["wc"] = _retile_rows(Bd[:, sh][:, 0:WCW], KT * 128).astype(BF16)

        in_shard = (uc_all >= j * NCOL) & (uc_all < (j + 1) * NCOL)
        ur = ur_all[in_shard]
        uc = uc_all[in_shard] - j * NCOL
        hv = uvals["H"][in_shard]
        av = uvals["A"][in_shard]

        # remainder cols 384:393 of B and H ride in the dense wcd3 tile
        cd3 = np.zeros((N, 41), np.float32)
        cd3[:, 0:9] = Bd[:, sh][:, WCW:NCOL]
        rem = uc >= WDW
        cd3[ur[rem], 32 + uc[rem] - WDW] = hv[rem]
        m["wcd3"] = _retile_rows(cd3, KT * 128).astype(BF16)

        # H chunk >= SCAT_WD rides as a dense DMA tile instead of a scatter
        kt_lim = SCAT_WD * CHUNK * 128
        hs = (~rem) & (ur < kt_lim)
        hidx, hval, need_h = _pack_scatter(ur[hs], uc[hs], hv[hs], WDW, nidx_cap)
        aidx, aval, need_a = _pack_scatter(ur, uc, av, WAW, nidx_cap)
        needed_max = max(needed_max, need_h, need_a)
        h4 = (~rem) & (ur >= kt_lim)
        wd4 = np.zeros(((KT - SCAT_WD * CHUNK) * 128, WDW), np.float32)
        wd4[ur[h4] - kt_lim, uc[h4]] = hv[h4]
        m["wd4"] = np.ascontiguousarray(
            wd4.reshape(KT - SCAT_WD * CHUNK, 128, WDW).transpose(1, 0, 2)
        ).astype(BF16)

        # hdr = idx lists | val lists (bf16 as i16) | mu/nu scalars
        idx2 = np.stack([hidx, aidx], 1).reshape(128, -1)  # [128, 2*NCH*nidx]
        val2 = (
            np.stack([hval, aval], 1).astype(BF16).view(np.int16)
            .reshape(128, -1)
        )
        m["hdr"] = np.ascontiguousarray(
            np.concatenate([idx2, val2, scb.view(np.int16)], axis=1)
        )

        uzcv = uz.copy()
        uzcv[:, 2 * TG :] = covf[:, sh]
        m["uzcv"] = uzcv.astype(BF16)
        msh = np.zeros((NMOB, T, MQ * 128), np.float32)
        msh[:, :, :NCOL] = Mf[:, :, sh]
        m["ms"] = np.ascontiguousarray(
            msh.reshape(NMOB, T, MQ, 128).transpose(3, 0, 2, 1)
        ).astype(BF16)
        in_maps.append(m)
    return in_maps, needed_max


def kernel(C, D, M, cov, B_nonzero, A_nonzero, H_nonzero, mu, nu, upsilon,
           zeta, rows, cols, **run_kwargs):
    nidx = NIDX_MIN
    in_maps, needed = _host_inputs(C, D, M, cov, B_nonzero, A_nonzero,
                                   H_nonzero, mu, nu, upsilon, zeta, rows,
                                   cols, nidx)
    if needed > nidx:
        # pathological inputs: rebuild with a large-enough (even) capacity
        nidx = (needed + 3) // 4 * 4
        in_maps, needed = _host_inputs(C, D, M, cov, B_nonzero, A_nonzero,
                                       H_nonzero, mu, nu, upsilon, zeta,
                                       rows, cols, nidx)
    nc = _get_program(nidx)
    res = run_bass_kernel_spmd(nc, in_maps, core_ids=list(range(NSH)), **run_kwargs)
    C_hat = np.concatenate(
        [res.results[j]["oc"][:NCOL].astype(np.float32).T for j in range(NSH)],
        axis=1,
    )
    D_hat = np.concatenate(
        [res.results[j]["od"][:NCOL].astype(np.float32).T for j in range(NSH)],
        axis=1,
    )
    if run_kwargs:
        kernel.last_results = res
    return C_hat.astype(np.float32), D_hat.astype(np.float32)
